# revision 4
# baseline (speedup 1.0000x reference)
"""AudioEncoder Trainium2 kernel, v3 — [feature, position] layout, stacked streams.

Computes: conv1d(1->64, k=5, stride=2, pad=2) + bias -> ReLU -> per-timestep
linear (64->64) + bias, over audio [4, 480000] f32 -> out [4, 240000, 64] f32.

Strategy (pure data parallel over 8 cores, S = 120000 output positions/core):
  - Each core splits its S positions into two streams (halves): stream A =
    positions [0, S/2) -> partitions 0-63 of everything downstream, stream B =
    [S/2, S) -> partitions 64-127.  All on-chip tensors are [feature(+stream),
    position], so every DMA is contiguous per partition.
  - Host pre-pads/casts audio to bf16 and de-interleaves to xe/xo streams so
    the 5 im2col rows per stream are contiguous reads (tap order 0,2,4,1,3).
    im2col tile rows: 0-4 = stream A (xe+0/1/2, xo+0/1), 5-9 = stream B.
  - Conv: ONE K=10, M=128 matmul per chunk.  Stationary wc10 [10,128] has
    wc in rows 0-4 x cols 0-63 (A) and rows 5-9 x cols 64-127 (B), zeros
    elsewhere, so both streams compute in a single full-width matmul.
  - Linear: ONE K=128, M=128 matmul per chunk with block-diagonal stationary
    [[w2,0],[0,w2]] -- streams stay separate, full PE array engaged (FWL).
  - Weights ping-pong (conv/linear) via the PE background weight buffer;
    chunks are processed in pairs per weight load (2 matmuls per LDWEIGHTS).
  - ACT evacuates conv PSUM (ReLU + conv bias) -> bf16 feats; DVE evacuates
    linear PSUM (+ linear bias) -> bf16 output tile (every 11th pair's linear
    evac goes to ACT to balance engine load).  Evacs read 2 PSUM banks per
    instruction via 3D strided APs.
  - im2col loads go through the Scalar-engine HWDGE queue, stores through the
    Sync queue, so loads are not FIFO-blocked behind multi-MB stores.
  - Output DRAM layout [128, S/2] bf16 (row = (stream, feature)); host
    transposes to [S, 64] and upcasts to f32.
"""

import numpy as np
import ml_dtypes

import concourse.bacc as bacc
import concourse.bass as bass
import concourse.mybir as mybir
import concourse.tile as tile
from concourse.bass_utils import run_bass_kernel_spmd

B = 4
T = 480000
S_FULL = 240000  # conv output positions per batch row
N_CORES = 8
S_CORE = S_FULL * B // N_CORES  # 120000 positions per core
E = 64  # conv out channels
P = 64  # linear out features
KS = 5
CH = 500  # positions per chunk (one PSUM bank holds <=512 f32)

f16 = mybir.dt.float16
f32 = mybir.dt.float32
bf16 = mybir.dt.bfloat16

DEF_SW = 4000  # store tile columns


def emit(nc: bass.Bass, S: int = S_CORE, WS=None, SW: int = DEF_SW) -> None:
    """Emit the per-core Tile kernel for S output positions (S/2 per stream)."""
    from contextlib import ExitStack

    H = S // 2
    if WS is None:
        # small first super-tile so the first matmul starts early
        WS = [4000, 28000, 28000]
    assert sum(WS) == H
    for W in WS:
        assert W % (2 * CH) == 0 and W % SW == 0
    assert SW % (2 * CH) == 0
    Wmax = max(WS)

    xs_d = nc.declare_dram_parameter("xs", [4, H + 2], bf16, isOutput=False)
    wc_d = nc.declare_dram_parameter("wc", [2 * KS, 128], bf16, isOutput=False)
    cb_d = nc.declare_dram_parameter("cb", [128, 1], f32, isOutput=False)
    w2_d = nc.declare_dram_parameter("w2", [128, 128], bf16, isOutput=False)
    lb_d = nc.declare_dram_parameter("lb", [128, 1], f32, isOutput=False)
    out_d = nc.declare_dram_parameter("out", [128, H], bf16, isOutput=True)

    RELU = mybir.ActivationFunctionType.Relu
    IDENT = mybir.ActivationFunctionType.Identity

    with tile.TileContext(nc) as tc, ExitStack() as ctx:
        consts = ctx.enter_context(tc.tile_pool(name="consts", bufs=1))
        imp = ctx.enter_context(tc.tile_pool(name="im", bufs=2))
        fpool = ctx.enter_context(tc.tile_pool(name="feats", bufs=4))
        opool = ctx.enter_context(tc.tile_pool(name="outs", bufs=3))
        pcp = ctx.enter_context(tc.tile_pool(name="psc", bufs=2, space="PSUM"))
        plp = ctx.enter_context(tc.tile_pool(name="psl", bufs=2, space="PSUM"))

        WC = consts.tile([2 * KS, 128], bf16)
        nc.scalar.dma_start(out=WC[:, :], in_=wc_d[:, :])
        W2 = consts.tile([128, 128], bf16)
        nc.scalar.dma_start(out=W2[:, :], in_=w2_d[:, :])
        CB = consts.tile([128, 1], f32)
        nc.scalar.dma_start(out=CB[:, :], in_=cb_d[:, :])
        LB = consts.tile([128, 1], f32)
        nc.scalar.dma_start(out=LB[:, :], in_=lb_d[:, :])

        outt = None
        ocol = 0
        pair_idx = 0

        def emit_loads(sc):
            # stream A rows 0-4 (xe+0/1/2, xo+0/1), stream B rows 5-9.
            # Loads on the Scalar HWDGE queue (stores go via Sync).
            W = WS[sc]
            sb = sum(WS[:sc])
            im = imp.tile([2 * KS, Wmax], bf16)
            for row, (xr, nr) in enumerate([(0, 3), (1, 2), (2, 3), (3, 2)]):
                o = sum(n for _, n in [(0, 3), (1, 2), (2, 3), (3, 2)][:row])
                nc.scalar.dma_start(
                    out=im[o : o + nr, 0:W],
                    in_=bass.AP(
                        tensor=xs_d, offset=xr * (H + 2) + sb, ap=[[1, nr], [1, W]]
                    ),
                )
            return im

        # prefetch: both im slots are free at kernel start, so issue the
        # first TWO supers' loads before any compute occupies the Scalar
        # queue; each later super's loads issue one full super ahead.
        im_tiles = {0: emit_loads(0)}
        if len(WS) > 1:
            im_tiles[1] = emit_loads(1)

        sbase = 0
        for sc, W in enumerate(WS):
            if sc > 0:
                sbase += WS[sc - 1]
            if sc >= 1 and sc + 1 < len(WS):
                im_tiles[sc + 1] = emit_loads(sc + 1)
            im = im_tiles.pop(sc)

            for pr in range(W // (2 * CH)):
                base = pr * 2 * CH
                if outt is None:
                    outt = opool.tile([128, SW], bf16)
                    ocol = 0

                psc = pcp.tile([128, 1024], f32)  # 2 banks, 2 chunks
                for q in (0, 1):
                    c = base + q * CH
                    # both streams in one K=10, M=128 matmul
                    nc.tensor.matmul(
                        out=psc[:, 512 * q : 512 * q + CH],
                        lhsT=WC[:, :],
                        rhs=im[:, c : c + CH],
                        start=True, stop=True,
                    )

                feats = fpool.tile([128, 1024], bf16)
                pscv = psc[:, :].rearrange("p (b c) -> p b c", b=2)[:, :, 0:CH]
                featv = feats[:, :].rearrange("p (b c) -> p b c", b=2)[:, :, 0:CH]
                nc.scalar.activation(
                    out=featv, in_=pscv, func=RELU, bias=CB[:, 0:1], scale=1.0
                )

                psl = plp.tile([128, 1024], f32)
                for q in (0, 1):
                    # block-diagonal K=128, M=128 matmul: both streams at once
                    nc.tensor.matmul(
                        out=psl[:, 512 * q : 512 * q + CH],
                        lhsT=W2[:, :],
                        rhs=feats[:, 512 * q : 512 * q + CH],
                        start=True, stop=True,
                    )

                pslv = psl[:, :].rearrange("p (b c) -> p b c", b=2)[:, :, 0:CH]
                outv = outt[:, ocol : ocol + 2 * CH].rearrange(
                    "p (b c) -> p b c", b=2
                )
                if pair_idx % 11 == 10:
                    # balance: occasional linear evac on ACT instead of DVE
                    nc.scalar.activation(
                        out=outv, in_=pslv, func=IDENT, bias=LB[:, 0:1], scale=1.0
                    )
                else:
                    nc.vector.tensor_scalar_add(out=outv, in0=pslv, scalar1=LB[:, 0:1])
                pair_idx += 1
                ocol += 2 * CH

                if ocol == SW:
                    gbase = sbase + (pr + 1) * 2 * CH - SW
                    nc.sync.dma_start(
                        out=out_d[:, gbase : gbase + SW], in_=outt[:, :]
                    )
                    outt = None


def prep_shared(conv_w, conv_b, lin_w, lin_b):
    """Host-side prep of the (tiny, replicated) parameter tensors."""
    conv_w = np.asarray(conv_w, dtype=np.float32)
    conv_b = np.asarray(conv_b, dtype=np.float32)
    lin_w = np.asarray(lin_w, dtype=np.float32)
    lin_b = np.asarray(lin_b, dtype=np.float32)

    wk = conv_w[:, 0, :]  # [64, 5]
    wc = wk[:, [0, 2, 4, 1, 3]].T  # [5, 64]
    wc10 = np.zeros((2 * KS, 128), dtype=ml_dtypes.bfloat16)
    wc10[0:KS, 0:E] = wc
    wc10[KS : 2 * KS, E : 2 * E] = wc
    cb = np.ascontiguousarray(
        np.concatenate([conv_b, conv_b]).astype(np.float32)[:, None]
    )  # [128, 1]
    w2 = lin_w.T  # [64e, 64p]
    w2blk = np.zeros((128, 128), dtype=ml_dtypes.bfloat16)
    w2blk[0:E, 0:P] = w2
    w2blk[E : 2 * E, P : 2 * P] = w2
    lb = np.ascontiguousarray(
        np.concatenate([lin_b, lin_b]).astype(np.float32)[:, None]
    )  # [128, 1]
    return wc10, cb, w2blk, lb


def prep_inputs(audio_waveform, conv_w, conv_b, lin_w, lin_b, S=S_CORE, n_cores=N_CORES):
    """Host-side shard + dtype/layout prep. Returns in_maps for the cores."""
    x = np.asarray(audio_waveform, dtype=np.float32)
    H = S // 2
    xp = np.zeros((x.shape[0], x.shape[1] + 4), dtype=ml_dtypes.bfloat16)
    xp[:, 2 : 2 + x.shape[1]] = x.astype(ml_dtypes.bfloat16)
    xe = xp[:, 0::2]
    xo = xp[:, 1::2]

    wc10, cb, w2blk, lb = prep_shared(conv_w, conv_b, lin_w, lin_b)

    in_maps = []
    for c in range(n_cores):
        b_i, half = divmod(c, 2)
        p0 = half * S
        xs = np.empty((4, H + 2), dtype=ml_dtypes.bfloat16)
        xs[0] = xe[b_i, p0 : p0 + H + 2]
        xs[1] = xo[b_i, p0 : p0 + H + 2]
        xs[2] = xe[b_i, p0 + H : p0 + 2 * H + 2]
        xs[3] = xo[b_i, p0 + H : p0 + 2 * H + 2]
        in_maps.append(dict(xs=xs, wc=wc10, cb=cb, w2=w2blk, lb=lb))
    return in_maps


_NC_CACHE = None


def get_nc() -> bass.Bass:
    global _NC_CACHE
    if _NC_CACHE is None:
        nc = bacc.Bacc()
        emit(nc)
        nc.compile()
        _NC_CACHE = nc
    return _NC_CACHE


def run(inputs: dict, trace: bool = False):
    """Run on the 8 cores; returns (full_output, BassKernelResults)."""
    in_maps = prep_inputs(**inputs)
    nc = get_nc()
    res = run_bass_kernel_spmd(nc, in_maps, list(range(N_CORES)), trace=trace)
    H = S_CORE // 2
    out = np.empty((B, S_FULL, P), dtype=np.float32)
    for c in range(N_CORES):
        b_i, half = divmod(c, 2)
        p0 = half * S_CORE
        o = np.asarray(res.results[c]["out"]).astype(np.float32)  # [128, H]
        out[b_i, p0 : p0 + H, :] = o[0:P, :].T
        out[b_i, p0 + H : p0 + 2 * H, :] = o[P : 2 * P, :].T
    return out, res


def kernel(**inputs) -> np.ndarray:
    out, _ = run(inputs)
    return out
